# revision 3
# baseline (speedup 1.0000x reference)
"""NeighborAttention (B=4, N=4096, K=32, C=128, H=4) on 8 Trainium2 cores.

v4 design (neighbor compaction + engine rebalance):

  Host packs each node's unmasked neighbors first (attention is
  permutation-invariant over neighbors), sorts nodes globally by
  neighbor count, and deals them round-robin to the 8 cores. Chunks of
  CN=512 nodes get a per-chunk neighbor capacity K' (multiple of 2,
  adaptive, typically [16,16,20,32]) so ~36% of all per-neighbor work
  and DMA disappears versus uniform K=32.

  Per chunk (j-major layout [c, j, n], bf16):
    A: PE kt_j = Wk'@et_j; DVE prod_j = kt_j*q  (bf16 out)
    B: PE s_jp = Hrep@prod (pairs); ACT e_jp = exp(s_jp) (bf16)
    Z: PE z = sum_j I@e_j (PSUM accumulate; replicated over partitions)
    C: PE vt_j = Wv'@et_j; DVE uv_j = e_j*vt_j (bf16)
    D: PE usum = sum_j I@uv_j (PSUM accumulate)
    T: DVE in-place max tree over uv slabs -> umax
    E: ACT copies (z->f32, usum->bf16); DVE zc=z-mc, clamp, rz=1/zc;
       PE o = Wos@usum + Wo3@umax; DVE out = o*rz  (softmax 1/z scaling
       commutes with the output projection, so it is applied once at
       the end); DMA out.

  attn sums to exactly 1 so the mean/sum W_O blocks fold on the host;
  compaction pad slots have et=0 and contribute exp(0)=1 to z,
  corrected by the host-sent (K' - count) term.
"""
import numpy as np
import ml_dtypes
import concourse.bass as bass
import concourse.bacc as bacc
import concourse.mybir as mybir
from concourse import tile
from concourse.bass_utils import run_bass_kernel_spmd

F32 = mybir.dt.float32
BF16 = mybir.dt.bfloat16
EXP = mybir.ActivationFunctionType.Exp

K = 32
C = 128
H = 4
D = 32
NCORES = 8
CN = 512              # nodes per chunk

_NC_CACHE = {}


def build_nc(sched):
    """sched: tuple of per-chunk neighbor capacities (even ints)."""
    key = tuple(sched)
    if key in _NC_CACHE:
        return _NC_CACHE[key]
    nchunks = len(sched)
    nloc = nchunks * CN
    ncols = sum(sched) * CN
    offs = np.cumsum([0] + [k * CN for k in sched]).tolist()

    nc = bacc.Bacc()
    et_d = nc.dram_tensor("et", [C, ncols], BF16, kind="ExternalInput")
    xt_d = nc.dram_tensor("xt", [C, nloc], BF16, kind="ExternalInput")
    wq_d = nc.dram_tensor("wq", [C, C], BF16, kind="ExternalInput")
    wk_d = nc.dram_tensor("wk", [C, C], BF16, kind="ExternalInput")
    wv_d = nc.dram_tensor("wv", [C, C], BF16, kind="ExternalInput")
    hr_d = nc.dram_tensor("hr", [C, C], BF16, kind="ExternalInput")
    id_d = nc.dram_tensor("idn", [C, C], BF16, kind="ExternalInput")
    wos_d = nc.dram_tensor("wos", [C, C], BF16, kind="ExternalInput")
    wo3_d = nc.dram_tensor("wo3", [C, C], BF16, kind="ExternalInput")
    mc_d = nc.dram_tensor("mc", [C, nloc], BF16, kind="ExternalInput")
    out_d = nc.dram_tensor("out", [C, nloc], F32, kind="ExternalOutput")

    with tile.TileContext(nc) as tc:
        with tc.tile_pool(name="wts", bufs=1) as wpool, \
             tc.tile_pool(name="glob", bufs=1) as gpool, \
             tc.tile_pool(name="etp", bufs=2) as etpool, \
             tc.tile_pool(name="s1p", bufs=2) as s1pool, \
             tc.tile_pool(name="s2p", bufs=2) as s2pool, \
             tc.tile_pool(name="sm", bufs=2) as smpool, \
             tc.tile_pool(name="pkv", bufs=2, space="PSUM") as pkv, \
             tc.tile_pool(name="psc", bufs=2, space="PSUM") as psc, \
             tc.tile_pool(name="psz", bufs=1, space="PSUM") as psz, \
             tc.tile_pool(name="psu", bufs=1, space="PSUM") as psu:

            w_q = wpool.tile([C, C], BF16, tag="wq")
            w_k = wpool.tile([C, C], BF16, tag="wk")
            w_v = wpool.tile([C, C], BF16, tag="wv")
            w_h = wpool.tile([C, C], BF16, tag="wh")
            w_i = wpool.tile([C, C], BF16, tag="wi")
            w_os = wpool.tile([C, C], BF16, tag="wos")
            w_o3 = wpool.tile([C, C], BF16, tag="wo3")
            for t, dd in ((w_q, wq_d), (w_k, wk_d), (w_v, wv_d), (w_h, hr_d),
                          (w_i, id_d), (w_os, wos_d), (w_o3, wo3_d)):
                nc.sync.dma_start(t[:], dd[:])

            xt_sb = gpool.tile([C, nloc], BF16, tag="xt")
            nc.sync.dma_start(xt_sb[:], xt_d[:])
            mc_sb = gpool.tile([C, nloc], BF16, tag="mc")
            nc.sync.dma_start(mc_sb[:], mc_d[:])

            def load_et(ch):
                kj = sched[ch]
                et_sb = etpool.tile([C, kj * CN], BF16, tag="et")
                half = kj * CN // 2
                nc.sync.dma_start(et_sb[:, :half],
                                  et_d[:, offs[ch]:offs[ch] + half])
                nc.sync.dma_start(et_sb[:, half:],
                                  et_d[:, offs[ch] + half:offs[ch + 1]])
                return et_sb

            et_tiles = {0: load_et(0)}

            for ch in range(nchunks):
                kj = sched[ch]
                n0 = ch * CN
                et_sb = et_tiles.pop(ch)
                if ch + 1 < nchunks:
                    et_tiles[ch + 1] = load_et(ch + 1)

                # q for this chunk
                q_ps = psu.tile([C, CN], F32, tag="u")
                nc.tensor.matmul(q_ps[:], w_q[:], xt_sb[:, n0:n0 + CN],
                                 start=True, stop=True)
                q_sb = smpool.tile([C, CN], F32, tag="q")
                nc.scalar.copy(q_sb[:], q_ps[:])

                prod = s1pool.tile([C, kj * CN], BF16, tag="s1")
                e_ch = s2pool.tile([C, kj * CN], BF16, tag="s2")

                # A: k-projection + prod
                for j in range(kj):
                    kt = pkv.tile([C, CN], F32, tag="kv")
                    nc.tensor.matmul(kt[:], w_k[:],
                                     et_sb[:, j * CN:(j + 1) * CN],
                                     start=True, stop=True)
                    nc.vector.tensor_mul(prod[:, j * CN:(j + 1) * CN],
                                         kt[:], q_sb[:])

                # B: scores (pairs) + exp
                for jp in range(kj // 2):
                    j = 2 * jp
                    s_ps = psc.tile([C, 2 * CN], F32, tag="s")
                    nc.tensor.matmul(s_ps[:, :CN], w_h[:],
                                     prod[:, j * CN:(j + 1) * CN],
                                     start=True, stop=True)
                    nc.tensor.matmul(s_ps[:, CN:], w_h[:],
                                     prod[:, (j + 1) * CN:(j + 2) * CN],
                                     start=True, stop=True)
                    nc.scalar.activation(e_ch[:, j * CN:(j + 2) * CN],
                                         s_ps[:], EXP)

                # Z: z = sum_j e_j via identity accumulation
                z_ps = psz.tile([C, CN], F32, tag="z")
                for j in range(kj):
                    nc.tensor.matmul(z_ps[:], w_i[:],
                                     e_ch[:, j * CN:(j + 1) * CN],
                                     start=(j == 0), stop=(j == kj - 1))

                # C: v-projection + uv (reuses prod tile as uv)
                uv = prod
                for j in range(kj):
                    vt = pkv.tile([C, CN], F32, tag="kv")
                    nc.tensor.matmul(vt[:], w_v[:],
                                     et_sb[:, j * CN:(j + 1) * CN],
                                     start=True, stop=True)
                    nc.vector.tensor_mul(uv[:, j * CN:(j + 1) * CN],
                                         e_ch[:, j * CN:(j + 1) * CN], vt[:])

                # D: usum via identity accumulation
                u_ps = psu.tile([C, CN], F32, tag="u")
                for j in range(kj):
                    nc.tensor.matmul(u_ps[:], w_i[:],
                                     uv[:, j * CN:(j + 1) * CN],
                                     start=(j == 0), stop=(j == kj - 1))

                # T: in-place max tree over uv slabs
                w = kj
                while w > 1:
                    hw = w // 2
                    nc.vector.tensor_max(uv[:, :hw * CN], uv[:, :hw * CN],
                                         uv[:, hw * CN:2 * hw * CN])
                    if w % 2:
                        nc.vector.tensor_max(uv[:, :CN], uv[:, :CN],
                                             uv[:, (w - 1) * CN:w * CN])
                    w = hw

                # E: epilogue — scale aggregates by 1/z per head BEFORE W_O
                zc = smpool.tile([C, CN], F32, tag="zc")
                nc.vector.tensor_sub(zc[:], z_ps[:], mc_sb[:, n0:n0 + CN])
                nc.vector.tensor_scalar_max(zc[:], zc[:], 1e-20)
                rz = smpool.tile([C, CN], F32, tag="rz")
                nc.vector.reciprocal(rz[:], zc[:])

                wsn = smpool.tile([C, CN], BF16, tag="wsn")
                nc.vector.tensor_mul(wsn[:], u_ps[:], rz[:])
                mxn = smpool.tile([C, CN], BF16, tag="mxn")
                nc.vector.tensor_mul(mxn[:], uv[:, :CN], rz[:])

                o_ps = psc.tile([C, 2 * CN], F32, tag="s")
                nc.tensor.matmul(o_ps[:, :CN], w_os[:], wsn[:],
                                 start=True, stop=False)
                nc.tensor.matmul(o_ps[:, :CN], w_o3[:], mxn[:],
                                 start=False, stop=True)
                o_sb = smpool.tile([C, CN], F32, tag="osb")
                nc.scalar.copy(o_sb[:], o_ps[:, :CN])
                nc.sync.dma_start(out_d[:, n0:n0 + CN], o_sb[:])

    nc.compile()
    _NC_CACHE[key] = nc
    return nc


def _perm_dh(w):
    """torch-layout [cout=(h*32+d), cin] -> lhsT [cin, cout2=(4d+h)]"""
    wt = np.asarray(w).reshape(H, D, -1)
    return np.ascontiguousarray(np.transpose(wt, (2, 1, 0)).reshape(-1, H * D))


def _even_up(x):
    x = max(int(x), 2)
    return x + (x & 1)


def prep_inputs(h_X, h_E, mask_attn, W_Q, W_K, W_V, W_O):
    h_X = np.asarray(h_X, dtype=np.float32)
    h_E = np.asarray(h_E, dtype=np.float32)
    mask_attn = np.asarray(mask_attn)
    W_Q = np.asarray(W_Q, dtype=np.float32)
    W_K = np.asarray(W_K, dtype=np.float32)
    W_V = np.asarray(W_V, dtype=np.float32)
    W_O = np.asarray(W_O, dtype=np.float32)

    B, N, Kn, Cin = h_E.shape
    BN = B * N
    nloc = BN // NCORES
    nchunks = nloc // CN

    m = (mask_attn.reshape(BN, Kn) > 0)
    cnt = m.sum(axis=1)
    order = np.argsort(cnt, kind="stable")        # global sorted node ids
    gchunk = NCORES * CN                          # nodes per global chunk

    # adaptive per-chunk capacity (same for every core)
    sched = tuple(_even_up(cnt[order[(ci + 1) * gchunk - 1]])
                  for ci in range(nchunks))

    # neighbor compaction indices: unmasked neighbors first
    jsel = np.argsort(~m, axis=1, kind="stable")  # [BN, K] unmasked first
    msort = np.take_along_axis(m, jsel, axis=1)   # [BN, K] descending 1s

    bf = ml_dtypes.bfloat16
    wq = _perm_dh(W_Q / np.sqrt(D)).astype(bf)
    wk = _perm_dh(W_K).astype(bf)
    wv = _perm_dh(W_V).astype(bf)

    idx = np.arange(C)
    hh = idx % H
    hrep = (hh[:, None] == hh[None, :]).astype(bf)
    ident = np.eye(C, dtype=np.float32).astype(bf)

    wos = W_O[:, :C] + W_O[:, C:2 * C]
    wo3 = W_O[:, 2 * C:]
    wost = np.ascontiguousarray(
        wos.T.reshape(H, D, C).transpose(1, 0, 2).reshape(C, C)).astype(bf)
    wo3t = np.ascontiguousarray(
        wo3.T.reshape(H, D, C).transpose(1, 0, 2).reshape(C, C)).astype(bf)

    hE = h_E.reshape(BN, Kn, Cin)
    xf = h_X.reshape(BN, -1)

    in_maps = []
    for core in range(NCORES):
        nid = order[core::NCORES]                 # this core's nodes, sorted
        et_parts = []
        for ch in range(nchunks):
            kj = sched[ch]
            nd = nid[ch * CN:(ch + 1) * CN]       # [CN]
            sel = jsel[nd][:, :kj]                # [CN, kj]
            g = hE[nd[:, None], sel]              # [CN, kj, Cin]
            g = g * msort[nd][:, :kj, None]       # zero pads
            # [CN, kj, Cin] -> [Cin, kj, CN]
            et_parts.append(g.transpose(2, 1, 0).reshape(Cin, kj * CN))
        etc = np.ascontiguousarray(np.concatenate(et_parts, axis=1)).astype(bf)
        xtc = np.ascontiguousarray(xf[nid].T).astype(bf)
        mcv = np.concatenate(
            [np.full(CN, sched[ch], np.float32) - cnt[nid[ch * CN:(ch + 1) * CN]]
             for ch in range(nchunks)])
        mcc = np.ascontiguousarray(
            np.broadcast_to(mcv, (C, nloc)).astype(bf))
        in_maps.append({
            "et": etc, "xt": xtc, "wq": wq, "wk": wk, "wv": wv,
            "hr": hrep, "idn": ident, "wos": wost, "wo3": wo3t, "mc": mcc,
        })
    return in_maps, sched, order


def assemble_output(results, B, N, order):
    BN = B * N
    nloc = BN // NCORES
    outf = np.empty((BN, C), np.float32)
    for core, r in enumerate(results):
        outf[order[core::NCORES]] = r["out"].T
    return outf.reshape(B, N, C)


def kernel(h_X, h_E, mask_attn, W_Q, W_K, W_V, W_O):
    in_maps, sched, order = prep_inputs(
        h_X, h_E, mask_attn, W_Q, W_K, W_V, W_O)
    nc = build_nc(sched)
    res = run_bass_kernel_spmd(nc, in_maps, core_ids=list(range(NCORES)))
    B, N = np.asarray(h_X).shape[:2]
    return assemble_output(res.results, B, N, order)
